# revision 27
# baseline (speedup 1.0000x reference)
"""Multi-head self-attention (pre-LN, residual) Trainium2 Bass kernel.

Problem: B=4, S=2048, D=128, H=4, Dh=32, fp32.
Sharding: 8 cores = 4 batches x 2 query-halves (1024 queries/core).
Each core receives its batch's full x, row-shuffled by the host so that
(a) the core's query half occupies device positions 0..1023 (attention is
permutation-invariant over keys) and (b) each SBUF partition loads
consecutive DRAM rows (8KB-contiguous DMA chunks at full bandwidth).
The host also ships the query-half of x pre-transposed (xqT) so the
residual path needs no PE transposes.

Fully transposed dataflow ([feature, seq] layouts) so the softmax
reduction rides the PE and no giant P-matrix transpose is needed:
  xn0^T --W--> Q^T,K^T [hd, s] bf16;  V [s, hd] bf16
  S^T[k,q] = K^T.T @ Q^T     2+2 heads packed via PE row-tiling (K=32)
  P_A = exp(S^T - 8)         heads {0,2} on ACT (table exp, bf16 out)
  P_B = schraudolph(S^T - 8) heads {1,3} on DVE: ONE tensor_scalar
                             (x*SA+SB) with int16 convert-on-write whose
                             bits are bf16 exp (min-RMS corrected, ~2%)
  ctx^T[hd,q] = V.T @ P      4 heads packed via PE col-tiling (M=32)
  den[hd,q]   = 1.T @ P      col-tiled ones-matmul (per-head row blocks)
  out^T = Wo.T @ (ctx^T * recip_approx(den)) + (xq^T + bias)
gamma/beta/all biases are folded into projection weights / per-partition
bias columns.  QKV/out projections run as float32r (tf32-like); scores
and P-side matmuls in bf16; all PSUM accumulation fp32.

Engine budget notes (empirical): exp on [128,1024] costs ~1335ns (ACT)
/ ~1469ns (DVE) and both engines total ~65us if they also carry the
elementwise prep work, so: xn-normalize and the residual add run on the
otherwise-idle GPSIMD engine (SBUF-only ops), PSUM->SBUF evacuations of
transposes / V-projection are batched 4 tiles -> one [128,512] copy,
and the input DMA is spread over four hardware queues with LN stats
emitted per-chunk.  Scores PSUM is triple-buffered (tag "s", 6 banks) +
ctx (1) + den (1) so the PE never waits on the scores->exp round trip.
The PE activity monitor (HAM) ignores transpose-mode matmuls and
re-throttles the clock to 1.2GHz after ~3.4us without dense activity:
zero-weight keeper matmuls (accumulating exact 0s into the den bank)
fire every other ktile, and the chunk0->chunk1 transition emits the
next chunk's first scores before the previous chunk's tail to avoid a
PE bubble.
"""

import sys

if "/opt/trn_rl_repo" not in sys.path:
    sys.path.insert(0, "/opt/trn_rl_repo")

import numpy as np

import concourse.bacc as bacc
import concourse.tile as tile
import concourse.mybir as mybir
from concourse.bass_utils import run_bass_kernel_spmd
from concourse.masks import make_identity

F32 = mybir.dt.float32
F32R = mybir.dt.float32r
BF16 = mybir.dt.bfloat16
I16 = mybir.dt.int16
AF = mybir.ActivationFunctionType
OP = mybir.AluOpType

B, S, D = 4, 2048, 128
H, DH = 4, 32
N_CORES = 8
QH = S // 2  # queries per core
NT = S // 128  # 16 s-tiles
CHUNK = 512
NCH = QH // CHUNK  # q-chunks per core
NKT = S // 128  # k-tiles
EPS = 1e-6
SHIFT = 8.0
ISQ = 1.0 / np.sqrt(np.float32(DH))
# Schraudolph bf16 exp: int16(x*SA + SB).bits == bf16(exp(x - SHIFT))
SA = float(128.0 / np.log(2.0))
SB = float(127.0 * 128.0 - 0.0579 * 128.0 - SHIFT * 128.0 / np.log(2.0))

GROUPS = ((0, 2), (1, 3))  # (A on ACT, B on DVE)
KEEP_EVERY = 2  # HAM keeper cadence (ktiles)

_compiled = None


def _build():
    nc = bacc.Bacc(
        "TRN2",
        target_bir_lowering=False,
        debug=False,
        enable_asserts=False,
        num_devices=N_CORES,
    )

    xkv_d = nc.dram_tensor("xkv", [S, D], F32, kind="ExternalInput").ap()
    xqT_d = nc.dram_tensor("xqT", [D, QH], F32, kind="ExternalInput").ap()
    wq_d = nc.dram_tensor("wq", [D, D], F32, kind="ExternalInput").ap()
    wk_d = nc.dram_tensor("wk", [D, D], F32, kind="ExternalInput").ap()
    wv_d = nc.dram_tensor("wv", [D, D], F32, kind="ExternalInput").ap()
    wo_d = nc.dram_tensor("wo", [D, D], F32, kind="ExternalInput").ap()
    # rows: gamma, beta, bq, bk, bv, bo
    vecs_d = nc.dram_tensor("vecs", [6, D], F32, kind="ExternalInput").ap()
    outT_d = nc.dram_tensor("outT", [D, QH], F32, kind="ExternalOutput").ap()

    with tile.TileContext(nc) as tc:
        consts = tc.alloc_tile_pool(name="consts", bufs=1)
        sbW = tc.alloc_tile_pool(name="sbW", bufs=1)
        sbBig = tc.alloc_tile_pool(name="sbBig", bufs=1)
        sbTmp = tc.alloc_tile_pool(name="sbTmp", bufs=3)
        pPool = tc.alloc_tile_pool(name="pPool", bufs=4)
        ps = tc.alloc_tile_pool(name="ps", bufs=1, space="PSUM")

        ident = consts.tile([128, 128], F32)
        make_identity(nc, ident)
        nshift = consts.tile([128, 1], F32)
        nc.vector.memset(nshift, -SHIFT)
        epsc = consts.tile([128, 1], F32)
        nc.vector.memset(epsc, EPS)
        zeroc = consts.tile([128, 1], F32)
        nc.vector.memset(zeroc, 0.0)
        wsrc = consts.tile([128, 512], BF16)
        nc.vector.memset(wsrc, 0.5)
        wones = consts.tile([128, DH], BF16)
        nc.vector.memset(wones, 1.0)
        zker = consts.tile([128, DH], BF16)
        nc.vector.memset(zker, 0.0)

        # ---- input DMAs: xkv spread over 2 HW queues; small first chunks
        # so block-0 stats can start early ----
        xkv_sb = sbBig.tile([128, NT, 128], F32)
        xkv_r = xkv_d.rearrange("(p t) d -> p t d", t=NT)
        mv_all = sbBig.tile([128, NT, 2], F32)
        for lo, hi, eng in ((0, 2, nc.sync), (2, 4, nc.scalar),
                            (4, 10, nc.sync), (10, 16, nc.scalar)):
            eng.dma_start(out=xkv_sb[:, lo:hi, :], in_=xkv_r[:, lo:hi, :])

        wq_raw = sbW.tile([D, D], F32)
        wk_raw = sbW.tile([D, D], F32)
        wv_raw = sbW.tile([D, D], F32)
        wo_raw = sbW.tile([D, D], F32)
        nc.gpsimd.dma_start(out=wq_raw, in_=wq_d)
        nc.gpsimd.dma_start(out=wk_raw, in_=wk_d)
        nc.gpsimd.dma_start(out=wv_raw, in_=wv_d)
        nc.gpsimd.dma_start(out=wo_raw, in_=wo_d)
        smallT = sbW.tile([D, 6], F32)  # cols: gamma,beta,bq,bk,bv,bo
        nc.gpsimd.dma_start(out=smallT, in_=vecs_d.rearrange("v d -> d v"))
        xqT_sb = sbBig.tile([128, QH], F32)
        nc.gpsimd.dma_start(out=xqT_sb, in_=xqT_d)

        # chunk-0 ctx/den PSUM allocated early so HAM warm-up bursts have a
        # target (garbage is cleared by the kt==0 start=True matmuls).
        ctx_ps = ps.tile([128, CHUNK], F32, name="ctx0", tag="ctx", bufs=1)
        den_ps = ps.tile([128, CHUNK], F32, name="den0", tag="den", bufs=1)
        for i in range(8):
            tgt = den_ps if i % 2 else ctx_ps
            nc.tensor.matmul(tgt[0:DH, :], wones, wsrc, start=True, stop=True)

        # preload the rsqrt ACT table during the DMAs
        tl0 = sbTmp.tile([128, 1], F32, tag="tl")
        nc.scalar.activation(tl0, epsc, AF.Abs_reciprocal_sqrt, bias=zeroc, scale=1.0)

        # ---- fold gamma/beta/biases ----
        gam = smallT[:, 0:1]
        bet = smallT[:, 1:2]
        gq = sbW.tile([128, 1], F32)
        nc.vector.tensor_scalar_mul(gq, gam, float(ISQ))
        wq_f = sbW.tile([D, D], F32R)
        wk_f = sbW.tile([D, D], F32R)
        wv_f = sbW.tile([D, D], F32R)
        nc.vector.tensor_scalar_mul(wq_f, wq_raw, gq)
        nc.vector.tensor_scalar_mul(wk_f, wk_raw, gam)
        nc.vector.tensor_scalar_mul(wv_f, wv_raw, gam)

        wo_r = sbW.tile([D, D], F32R)
        nc.vector.tensor_copy(wo_r, wo_raw)
        bqe = sbW.tile([128, 1], F32)
        bke = sbW.tile([128, 1], F32)
        bve = sbW.tile([128, 1], F32)
        rbias = sbW.tile([128, 1], F32)
        t_ps = ps.tile([128, 1], F32, name="t_ps", tag="s", bufs=3)
        nc.tensor.matmul(t_ps, wq_raw, bet, start=True, stop=True)
        nc.vector.tensor_scalar(
            bqe, t_ps, smallT[:, 2:3], float(ISQ), op0=OP.add, op1=OP.mult
        )
        t_ps = ps.tile([128, 1], F32, name="t_ps", tag="s", bufs=3)
        nc.tensor.matmul(t_ps, wk_raw, bet, start=True, stop=True)
        nc.vector.tensor_scalar_add(bke, t_ps, smallT[:, 3:4])
        t_ps = ps.tile([128, 1], F32, name="t_ps", tag="s", bufs=3)
        nc.tensor.matmul(t_ps, wv_raw, bet, start=True, stop=True)
        nc.vector.tensor_scalar_add(bve, t_ps, smallT[:, 4:5])
        t_ps = ps.tile([128, 1], F32, name="t_ps", tag="s", bufs=3)
        nc.tensor.matmul(t_ps, wo_raw, bve, start=True, stop=True)
        nc.vector.tensor_scalar_add(rbias, t_ps, smallT[:, 5:6])

        # ---- block-pipelined prep: each 4-tile block flows
        # stats -> rsqrt -> xn -> transpose -> projections independently,
        # chasing its DMA chunk ----
        lnv = sbBig.tile([128, NT], F32)
        rs_all = sbBig.tile([128, NT], F32)
        xn0_sb = sbBig.tile([128, NT, 128], F32)
        xkvT = sbBig.tile([128, S], F32R)  # xn0^T [d, s]
        kT = sbBig.tile([128, S], BF16)
        qT = sbBig.tile([128, QH], BF16)
        v_sb = sbBig.tile([128, NT, 128], BF16)
        residT = sbBig.tile([128, QH], F32)  # xq^T + resid_bias

        def keeper(tgt, start):
            # full-row-group zero-weight matmul: accumulates exact zeros,
            # registers as PE activity for the HAM clock gate.
            nc.tensor.matmul(
                tgt[0:DH, 0:256], zker, wsrc[:, 0:256], start=start, stop=start
            )

        def stats_block(b4):
            sl4 = slice(b4 * 4, b4 * 4 + 4)
            for t in range(b4 * 4, b4 * 4 + 4):
                stats = sbTmp.tile([128, 6], F32, tag="st")
                nc.vector.bn_stats(stats, xkv_sb[:, t, :])
                nc.vector.bn_aggr(mv_all[:, t, :], stats)
            nc.vector.tensor_scalar_add(lnv[:, sl4], mv_all[:, sl4, 1], epsc)

        def xn_block(b4):
            for t in range(b4 * 4, b4 * 4 + 4):
                nc.vector.tensor_scalar(
                    xn0_sb[:, t, :],
                    xkv_sb[:, t, :],
                    mv_all[:, t, 0:1],
                    rs_all[:, t : t + 1],
                    op0=OP.subtract,
                    op1=OP.mult,
                )

        def proj_block(b4):
            # PSUM evacuations alternate ACT/DVE so neither engine's
            # in-order queue serializes the prep tail
            tp = ps.tile([128, 512], F32, name="tp", tag="s", bufs=3)
            for j in range(4):
                t = b4 * 4 + j
                nc.tensor.transpose(
                    tp[:, j * 128 : (j + 1) * 128], xn0_sb[:, t, :], ident
                )
            if b4 % 2 == 0:
                nc.scalar.copy(xkvT[:, b4 * 512 : (b4 + 1) * 512], tp)
            else:
                nc.vector.tensor_copy(xkvT[:, b4 * 512 : (b4 + 1) * 512], tp)
            pp = ps.tile([128, CHUNK], F32, name="pp", tag="s", bufs=3)
            nc.tensor.matmul(
                pp, wk_f, xkvT[:, b4 * CHUNK : (b4 + 1) * CHUNK],
                start=True, stop=True,
            )
            nc.scalar.activation(
                kT[:, b4 * CHUNK : (b4 + 1) * CHUNK], pp, AF.Identity,
                bias=bke, scale=1.0,
            )
            if b4 < NCH:
                pp = ps.tile([128, CHUNK], F32, name="pp", tag="s", bufs=3)
                nc.tensor.matmul(
                    pp, wq_f, xkvT[:, b4 * CHUNK : (b4 + 1) * CHUNK],
                    start=True, stop=True,
                )
                if b4 == 0:
                    nc.scalar.activation(
                        qT[:, b4 * CHUNK : (b4 + 1) * CHUNK], pp, AF.Identity,
                        bias=bqe, scale=1.0,
                    )
                else:
                    nc.vector.tensor_scalar_add(
                        qT[:, b4 * CHUNK : (b4 + 1) * CHUNK], pp, bqe
                    )
            pv = ps.tile([128, 512], F32, name="pv", tag="s", bufs=3)
            for j in range(4):
                t = b4 * 4 + j
                nc.tensor.matmul(
                    pv[:, j * 128 : (j + 1) * 128],
                    xkvT[:, t * 128 : (t + 1) * 128],
                    wv_f,
                    start=True,
                    stop=True,
                )
            if b4 % 2 == 1:
                nc.scalar.copy(v_sb[:, b4 * 4 : (b4 + 1) * 4, :], pv)
            else:
                nc.vector.tensor_copy(v_sb[:, b4 * 4 : (b4 + 1) * 4, :], pv)
            keeper(ctx_ps, True)

        # upfront: only what chunk-0's first ktiles need (block 0), plus
        # all stats/rsqrt so the Exp table loads exactly once.  Blocks
        # 1-3 are emitted inside the attention loop so early exps sit
        # ahead of them in the in-order ACT queue.
        stats_block(0)
        nc.scalar.activation(
            rs_all[:, 0:4], lnv[:, 0:4], AF.Abs_reciprocal_sqrt,
            bias=zeroc, scale=1.0,
        )
        xn_block(0)
        for b4 in range(1, 4):
            stats_block(b4)
        proj_block(0)
        nc.scalar.activation(
            rs_all[:, 4:NT], lnv[:, 4:NT], AF.Abs_reciprocal_sqrt,
            bias=zeroc, scale=1.0,
        )
        # Rewrite nshift (= rs*0 - SHIFT) after the last rsqrt: every
        # attention exp reads nshift as its bias, so this data-dep stops the
        # scheduler from hoisting any Exp above the rsqrts (table thrash).
        nc.vector.tensor_scalar(
            nshift, rs_all[:, NT - 1 : NT], 0.0, -SHIFT, op0=OP.mult, op1=OP.add
        )
        # preload Exp table
        tl1 = sbTmp.tile([128, 1], F32, tag="tl")
        nc.scalar.activation(tl1, rs_all[:, NT - 1 : NT], AF.Exp, bias=nshift, scale=1.0)
        for b4 in range(1, 4):
            xn_block(b4)
            proj_block(b4)
        nc.vector.tensor_scalar_add(residT, xqT_sb, rbias)

        # ---- attention ----
        ctx_sb_unused = None  # ctx read straight from PSUM in the tail
        den_all = sbBig.tile([128, NCH, CHUNK], F32)

        def attn_scores(qc, kt):
            q0 = qc * CHUNK
            k0 = kt * 128
            p_sb = [None, None]
            for g, heads in enumerate(GROUPS):
                sp = ps.tile(
                    [128, 2 * CHUNK], F32, name=f"s{g}", tag="s", bufs=3
                )
                for i, h in enumerate(heads):
                    nc.tensor.matmul(
                        sp[:, i * CHUNK : (i + 1) * CHUNK],
                        kT[h * DH : (h + 1) * DH, k0 : k0 + 128],
                        qT[h * DH : (h + 1) * DH, q0 : q0 + CHUNK],
                        start=True,
                        stop=True,
                        tile_position=(h * DH, 0),
                    )
                if g == 0:
                    pA = pPool.tile([128, 2 * CHUNK], BF16, tag="pa")
                    nc.scalar.activation(pA, sp, AF.Exp, bias=nshift, scale=1.0)
                    p_sb[0] = pA
                else:
                    pB = pPool.tile([128, 2 * CHUNK], I16, tag="pb")
                    nc.vector.tensor_scalar(pB, sp, SA, SB, op0=OP.mult, op1=OP.add)
                    p_sb[1] = pB.bitcast(BF16)
            return p_sb

        def attn_ctxden(cps, dps, kt, p_sb):
            def ctx_mms():
                for g, heads in enumerate(GROUPS):
                    for i, h in enumerate(heads):
                        nc.tensor.matmul(
                            cps[h * DH : (h + 1) * DH, :],
                            v_sb[:, kt, h * DH : (h + 1) * DH],
                            p_sb[g][:, i * CHUNK : (i + 1) * CHUNK],
                            start=(kt == 0),
                            stop=(kt == NKT - 1),
                            tile_position=(0, h * DH),
                        )

            def den_mms():
                for g, heads in enumerate(GROUPS):
                    for i, h in enumerate(heads):
                        nc.tensor.matmul(
                            dps[h * DH : (h + 1) * DH, :],
                            wones,
                            p_sb[g][:, i * CHUNK : (i + 1) * CHUNK],
                            start=(kt == 0),
                            stop=(kt == NKT - 1),
                            tile_position=(0, h * DH),
                        )

            if kt == NKT - 1:
                # den first: the recip (critical path into the chunk tail)
                # starts while the last ctx matmuls still run
                den_mms()
                ctx_mms()
            else:
                ctx_mms()
                den_mms()

        def chunk_tail(qc, cps, halves=1):
            q0 = qc * CHUNK
            hw = CHUNK // halves
            ctxn = sbTmp.tile([128, CHUNK], F32R, tag="cn")
            out_ps = ps.tile([128, CHUNK], F32, name="out_ps", tag="s", bufs=3)
            fin = sbTmp.tile([128, CHUNK], F32, tag="fin")
            for hh in range(halves):
                sl = slice(hh * hw, (hh + 1) * hw)
                nc.vector.tensor_mul(ctxn[:, sl], cps[:, sl], den_all[:, qc, sl])
                nc.tensor.matmul(
                    out_ps[:, sl], wo_r, ctxn[:, sl], start=True, stop=True
                )
                nc.vector.tensor_add(
                    fin[:, sl], out_ps[:, sl], residT[:, q0 + hh * hw : q0 + (hh + 1) * hw]
                )
                nc.sync.dma_start(
                    out=outT_d[:, q0 + hh * hw : q0 + (hh + 1) * hw], in_=fin[:, sl]
                )

        # pre-attention warm burst (prep transposes don't register with HAM)
        for i in range(4):
            nc.tensor.matmul(ctx_ps[0:DH, :], wones, wsrc, start=True, stop=True)

        pending = attn_scores(0, 0)
        cur_ctx, cur_den = ctx_ps, den_ps
        for qc in range(NCH):
            if qc > 0:
                # allocated lazily AFTER this chunk's first scores so the
                # WAR-wait on the previous chunk's recip/ctxn readers lands
                # behind runnable PE work instead of stalling the LDW queue
                cur_ctx = ps.tile(
                    [128, CHUNK], F32, name=f"ctx{qc}", tag="ctx", bufs=1
                )
                cur_den = ps.tile(
                    [128, CHUNK], F32, name=f"den{qc}", tag="den", bufs=1
                )
            for kt in range(NKT):
                if kt + 1 < NKT:
                    nxt = attn_scores(qc, kt + 1)
                elif qc + 1 < NCH:
                    # next chunk's first scores BEFORE this chunk's tail so
                    # the PE stream has no bubble at the boundary
                    nxt = attn_scores(qc + 1, 0)
                else:
                    nxt = None
                attn_ctxden(cur_ctx, cur_den, kt, pending)
                if kt == 0 or (kt % KEEP_EVERY == 1 and kt < NKT - 1):
                    keeper(cur_den, False)
                pending = nxt
            if qc == NCH - 1:
                # final chunk: pipeline the tail in halves to shorten the
                # serial recip->mul->proj->add->DMA ramp-down
                nc.vector.reciprocal_approx_fast(
                    den_all[:, qc, 0:256], cur_den[:, 0:256]
                )
                nc.vector.reciprocal_approx_fast(
                    den_all[:, qc, 256:512], cur_den[:, 256:512]
                )
                chunk_tail(qc, cur_ctx, halves=2)
            else:
                nc.vector.reciprocal_approx_fast(den_all[:, qc, :], cur_den)
                chunk_tail(qc, cur_ctx)

        pPool.release()
        ps.release()
        sbTmp.release()
        sbBig.release()
        sbW.release()
        consts.release()

    nc.compile()
    return nc


def _get_compiled():
    global _compiled
    if _compiled is None:
        _compiled = _build()
    return _compiled


# device position j <- host row (j%128)*16 + j//128
_DEV2HOST = (np.arange(S) % 128) * NT + np.arange(S) // 128
_HOSTPERM = np.empty(S, dtype=np.int64)
_HOSTPERM[_DEV2HOST] = np.arange(S)


def kernel(x, Wq, bq, Wk, bk, Wv, bv, gamma, beta, Wo, bo):
    x = np.asarray(x, dtype=np.float32)
    vecs = np.stack(
        [np.asarray(a, dtype=np.float32) for a in (gamma, beta, bq, bk, bv, bo)]
    )
    wq = np.ascontiguousarray(np.asarray(Wq, dtype=np.float32))
    wk = np.ascontiguousarray(np.asarray(Wk, dtype=np.float32))
    wv = np.ascontiguousarray(np.asarray(Wv, dtype=np.float32))
    wo = np.ascontiguousarray(np.asarray(Wo, dtype=np.float32))

    nc = _get_compiled()

    in_maps = []
    for c in range(N_CORES):
        b, half = c // 2, c % 2
        off = half * QH
        xroll = np.roll(x[b], -off, axis=0)
        xin = np.ascontiguousarray(xroll[_HOSTPERM])
        xqT = np.ascontiguousarray(xroll[:QH].T)
        in_maps.append(
            {
                "xkv": xin,
                "xqT": xqT,
                "wq": wq,
                "wk": wk,
                "wv": wv,
                "wo": wo,
                "vecs": vecs,
            }
        )

    res = run_bass_kernel_spmd(nc, in_maps, core_ids=list(range(N_CORES)), trace=False)

    out = np.empty((B, S, D), dtype=np.float32)
    for c in range(N_CORES):
        b, half = c // 2, c % 2
        off = half * QH
        out[b, off : off + QH, :] = res.results[c]["outT"].T
    return out


# revision 32
# speedup vs baseline: 1.0686x; 1.0686x over previous
"""Multi-head self-attention (pre-LN, residual) Trainium2 Bass kernel.

Problem: B=4, S=2048, D=128, H=4, Dh=32, fp32.
Sharding: 8 cores = 4 batches x 2 query-halves (1024 queries/core).
Each core receives its batch's full x, row-shuffled by the host so that
(a) the core's query half occupies device positions 0..1023 (attention is
permutation-invariant over keys) and (b) each SBUF partition loads
consecutive DRAM rows (8KB-contiguous DMA chunks at full bandwidth).
The host also ships the query-half of x pre-transposed (xqT) so the
residual path needs no PE transposes.

Fully transposed dataflow ([feature, seq] layouts) so the softmax
reduction rides the PE and no giant P-matrix transpose is needed:
  xn0^T --W--> Q^T,K^T [hd, s] bf16;  V [s, hd] bf16
  S^T[k,q] = K^T.T @ Q^T     2+2 heads packed via PE row-tiling (K=32)
  P_A = exp(S^T - 8)         heads {0,2} on ACT (table exp, bf16 out)
  P_B = schraudolph(S^T - 8) heads {1,3} on DVE: ONE tensor_scalar
                             (x*SA+SB) with int16 convert-on-write whose
                             bits are bf16 exp (min-RMS corrected, ~2%)
  ctx^T[hd,q] = V.T @ P      4 heads packed via PE col-tiling (M=32)
  den[hd,q]   = 1.T @ P      col-tiled ones-matmul (per-head row blocks)
  out^T = Wo.T @ (ctx^T * recip_approx(den)) + (xq^T + bias)
gamma/beta/all biases are folded into projection weights / per-partition
bias columns.  QKV/out projections run as float32r (tf32-like); scores
and P-side matmuls in bf16; all PSUM accumulation fp32.

Engine budget notes (empirical): exp on [128,1024] costs ~1335ns (ACT)
/ ~1469ns (DVE) and both engines total ~65us if they also carry the
elementwise prep work, so: xn-normalize and the residual add run on the
otherwise-idle GPSIMD engine (SBUF-only ops), PSUM->SBUF evacuations of
transposes / V-projection are batched 4 tiles -> one [128,512] copy,
and the input DMA is spread over four hardware queues with LN stats
emitted per-chunk.  Scores PSUM is triple-buffered (tag "s", 6 banks) +
ctx (1) + den (1) so the PE never waits on the scores->exp round trip.
The PE activity monitor (HAM) ignores transpose-mode matmuls and
re-throttles the clock to 1.2GHz after ~3.4us without dense activity:
zero-weight keeper matmuls (accumulating exact 0s into the den bank)
fire every other ktile, and the chunk0->chunk1 transition emits the
next chunk's first scores before the previous chunk's tail to avoid a
PE bubble.
"""

import sys

if "/opt/trn_rl_repo" not in sys.path:
    sys.path.insert(0, "/opt/trn_rl_repo")

import numpy as np

import concourse.bacc as bacc
import concourse.tile as tile
import concourse.mybir as mybir
from concourse.bass_utils import run_bass_kernel_spmd
from concourse.masks import make_identity

F32 = mybir.dt.float32
F32R = mybir.dt.float32r
BF16 = mybir.dt.bfloat16
I16 = mybir.dt.int16
AF = mybir.ActivationFunctionType
OP = mybir.AluOpType

B, S, D = 4, 2048, 128
H, DH = 4, 32
N_CORES = 8
QH = S // 2  # queries per core
NT = S // 128  # 16 s-tiles
CHUNK = 512
NCH = QH // CHUNK  # q-chunks per core
NKT = S // 128  # k-tiles
EPS = 1e-6
SHIFT = 8.0
ISQ = 1.0 / np.sqrt(np.float32(DH))
# Schraudolph bf16 exp: int16(x*SA + SB).bits == bf16(exp(x - SHIFT))
SA = float(128.0 / np.log(2.0))
SB = float(127.0 * 128.0 - 0.0579 * 128.0 - SHIFT * 128.0 / np.log(2.0))

GROUPS = ((0, 2), (1, 3))  # (A on ACT, B on DVE)
KEEP_EVERY = 2  # HAM keeper cadence (ktiles)

_compiled = None


def _build():
    nc = bacc.Bacc(
        "TRN2",
        target_bir_lowering=False,
        debug=False,
        enable_asserts=False,
        num_devices=N_CORES,
    )

    xkv_d = nc.dram_tensor("xkv", [S, D], F32, kind="ExternalInput").ap()
    xqT_d = nc.dram_tensor("xqT", [D, QH], F32, kind="ExternalInput").ap()
    wq_d = nc.dram_tensor("wq", [D, D], F32, kind="ExternalInput").ap()
    wk_d = nc.dram_tensor("wk", [D, D], F32, kind="ExternalInput").ap()
    wv_d = nc.dram_tensor("wv", [D, D], F32, kind="ExternalInput").ap()
    wo_d = nc.dram_tensor("wo", [D, D], F32, kind="ExternalInput").ap()
    # rows: gamma, beta, bq, bk, bv, bo
    vecs_d = nc.dram_tensor("vecs", [6, D], F32, kind="ExternalInput").ap()
    outT_d = nc.dram_tensor("outT", [D, QH], F32, kind="ExternalOutput").ap()

    with tile.TileContext(nc) as tc:
        consts = tc.alloc_tile_pool(name="consts", bufs=1)
        sbW = tc.alloc_tile_pool(name="sbW", bufs=1)
        sbBig = tc.alloc_tile_pool(name="sbBig", bufs=1)
        sbTmp = tc.alloc_tile_pool(name="sbTmp", bufs=3)
        pPool = tc.alloc_tile_pool(name="pPool", bufs=3)
        ps = tc.alloc_tile_pool(name="ps", bufs=1, space="PSUM")

        ident = consts.tile([128, 128], F32)
        make_identity(nc, ident)
        nshift = consts.tile([128, 1], F32)
        nc.vector.memset(nshift, -SHIFT)
        epsc = consts.tile([128, 1], F32)
        nc.vector.memset(epsc, EPS)
        zeroc = consts.tile([128, 1], F32)
        nc.vector.memset(zeroc, 0.0)
        wsrc = consts.tile([128, 512], BF16)
        nc.vector.memset(wsrc, 0.5)
        wones = consts.tile([128, DH], BF16)
        nc.vector.memset(wones, 1.0)
        zker = consts.tile([128, DH], BF16)
        nc.vector.memset(zker, 0.0)

        # ---- input DMAs: xkv spread over 2 HW queues, stats per chunk ----
        xkv_sb = sbBig.tile([128, NT, 128], F32)
        xkv_r = xkv_d.rearrange("(p t) d -> p t d", t=NT)
        mv_all = sbBig.tile([128, NT, 2], F32)
        qeng = [nc.sync, nc.scalar, nc.sync, nc.scalar]
        for c4 in range(4):
            qeng[c4].dma_start(
                out=xkv_sb[:, c4 * 4 : (c4 + 1) * 4, :],
                in_=xkv_r[:, c4 * 4 : (c4 + 1) * 4, :],
            )
            for t in range(c4 * 4, c4 * 4 + 4):
                stats = sbTmp.tile([128, 6], F32, tag="st")
                nc.vector.bn_stats(stats, xkv_sb[:, t, :])
                nc.vector.bn_aggr(mv_all[:, t, :], stats)

        wq_raw = sbW.tile([D, D], F32)
        wk_raw = sbW.tile([D, D], F32)
        wv_raw = sbW.tile([D, D], F32)
        wo_raw = sbW.tile([D, D], F32)
        nc.gpsimd.dma_start(out=wq_raw, in_=wq_d)
        nc.gpsimd.dma_start(out=wk_raw, in_=wk_d)
        nc.gpsimd.dma_start(out=wv_raw, in_=wv_d)
        nc.gpsimd.dma_start(out=wo_raw, in_=wo_d)
        smallT = sbW.tile([D, 6], F32)  # cols: gamma,beta,bq,bk,bv,bo
        nc.gpsimd.dma_start(out=smallT, in_=vecs_d.rearrange("v d -> d v"))
        xqT_sb = sbBig.tile([128, QH], F32)
        nc.gpsimd.dma_start(out=xqT_sb, in_=xqT_d)

        # chunk-0 ctx/den PSUM allocated early so HAM warm-up bursts have a
        # target (garbage is cleared by the kt==0 start=True matmuls).
        ctx_ps = ps.tile([128, CHUNK], F32, name="ctx0", tag="ctx", bufs=1)
        den_ps = ps.tile([128, CHUNK], F32, name="den0", tag="den", bufs=1)
        for i in range(8):
            tgt = den_ps if i % 2 else ctx_ps
            nc.tensor.matmul(tgt[0:DH, :], wones, wsrc, start=True, stop=True)

        # preload the rsqrt ACT table during the DMAs
        tl0 = sbTmp.tile([128, 1], F32, tag="tl")
        nc.scalar.activation(tl0, epsc, AF.Abs_reciprocal_sqrt, bias=zeroc, scale=1.0)

        # ---- fold gamma/beta/biases ----
        gam = smallT[:, 0:1]
        bet = smallT[:, 1:2]
        gq = sbW.tile([128, 1], F32)
        nc.vector.tensor_scalar_mul(gq, gam, float(ISQ))
        wq_f = sbW.tile([D, D], F32R)
        wk_f = sbW.tile([D, D], F32R)
        wv_f = sbW.tile([D, D], F32R)
        nc.vector.tensor_scalar_mul(wq_f, wq_raw, gq)
        nc.vector.tensor_scalar_mul(wk_f, wk_raw, gam)
        nc.vector.tensor_scalar_mul(wv_f, wv_raw, gam)

        wo_r = sbW.tile([D, D], F32R)
        nc.vector.tensor_copy(wo_r, wo_raw)
        bqe = sbW.tile([128, 1], F32)
        bke = sbW.tile([128, 1], F32)
        bve = sbW.tile([128, 1], F32)
        rbias = sbW.tile([128, 1], F32)
        t_ps = ps.tile([128, 1], F32, name="t_ps", tag="s", bufs=3)
        nc.tensor.matmul(t_ps, wq_raw, bet, start=True, stop=True)
        nc.vector.tensor_scalar(
            bqe, t_ps, smallT[:, 2:3], float(ISQ), op0=OP.add, op1=OP.mult
        )
        t_ps = ps.tile([128, 1], F32, name="t_ps", tag="s", bufs=3)
        nc.tensor.matmul(t_ps, wk_raw, bet, start=True, stop=True)
        nc.vector.tensor_scalar_add(bke, t_ps, smallT[:, 3:4])
        t_ps = ps.tile([128, 1], F32, name="t_ps", tag="s", bufs=3)
        nc.tensor.matmul(t_ps, wv_raw, bet, start=True, stop=True)
        nc.vector.tensor_scalar_add(bve, t_ps, smallT[:, 4:5])
        t_ps = ps.tile([128, 1], F32, name="t_ps", tag="s", bufs=3)
        nc.tensor.matmul(t_ps, wo_raw, bve, start=True, stop=True)
        nc.vector.tensor_scalar_add(rbias, t_ps, smallT[:, 5:6])

        # ---- block-pipelined prep: each 4-tile block flows
        # stats -> rsqrt -> xn -> transpose -> projections independently,
        # chasing its DMA chunk ----
        lnv = sbBig.tile([128, NT], F32)
        rs_all = sbBig.tile([128, NT], F32)
        xn0_sb = sbBig.tile([128, NT, 128], F32)
        xkvT = sbBig.tile([128, S], F32R)  # xn0^T [d, s]
        kT = sbBig.tile([128, S], BF16)
        qT = sbBig.tile([128, QH], BF16)
        v_sb = sbBig.tile([128, NT, 128], BF16)
        residT = sbBig.tile([128, QH], F32)  # xq^T + resid_bias

        def keeper(tgt, start):
            # full-row-group zero-weight matmul: accumulates exact zeros,
            # registers as PE activity for the HAM clock gate.
            nc.tensor.matmul(
                tgt[0:DH, 0:256], zker, wsrc[:, 0:256], start=start, stop=start
            )

        # ---- LayerNorm scale (single rsqrt batch, then pin Exp table) ----
        nc.vector.tensor_scalar_add(lnv, mv_all[:, :, 1], epsc)
        nc.scalar.activation(
            rs_all, lnv, AF.Abs_reciprocal_sqrt, bias=zeroc, scale=1.0
        )
        # Rewrite nshift (= rs*0 - SHIFT) after the rsqrt: every attention
        # exp reads nshift as its bias, so this data-dep stops the scheduler
        # from hoisting any Exp above the rsqrt (table thrash).
        nc.vector.tensor_scalar(
            nshift, rs_all[:, NT - 1 : NT], 0.0, -SHIFT, op0=OP.mult, op1=OP.add
        )
        # preload Exp table
        tl1 = sbTmp.tile([128, 1], F32, tag="tl")
        nc.scalar.activation(tl1, rs_all[:, NT - 1 : NT], AF.Exp, bias=nshift, scale=1.0)

        for t in range(NT):
            nc.vector.tensor_scalar(
                xn0_sb[:, t, :],
                xkv_sb[:, t, :],
                mv_all[:, t, 0:1],
                rs_all[:, t : t + 1],
                op0=OP.subtract,
                op1=OP.mult,
            )
        for b4 in range(4):
            tp = ps.tile([128, 512], F32, name="tp", tag="s", bufs=3)
            for j in range(4):
                t = b4 * 4 + j
                nc.tensor.transpose(
                    tp[:, j * 128 : (j + 1) * 128], xn0_sb[:, t, :], ident
                )
            nc.scalar.copy(xkvT[:, b4 * 512 : (b4 + 1) * 512], tp)
            keeper(ctx_ps, True)

        for c in range(4):
            pp = ps.tile([128, CHUNK], F32, name="pp", tag="s", bufs=3)
            nc.tensor.matmul(
                pp, wk_f, xkvT[:, c * CHUNK : (c + 1) * CHUNK], start=True, stop=True
            )
            nc.scalar.activation(
                kT[:, c * CHUNK : (c + 1) * CHUNK], pp, AF.Identity,
                bias=bke, scale=1.0,
            )
        for c in range(NCH):
            pp = ps.tile([128, CHUNK], F32, name="pp", tag="s", bufs=3)
            nc.tensor.matmul(
                pp, wq_f, xkvT[:, c * CHUNK : (c + 1) * CHUNK], start=True, stop=True
            )
            nc.scalar.activation(
                qT[:, c * CHUNK : (c + 1) * CHUNK], pp, AF.Identity,
                bias=bqe, scale=1.0,
            )
        for b4 in range(4):
            pv = ps.tile([128, 512], F32, name="pv", tag="s", bufs=3)
            for j in range(4):
                t = b4 * 4 + j
                nc.tensor.matmul(
                    pv[:, j * 128 : (j + 1) * 128],
                    xkvT[:, t * 128 : (t + 1) * 128],
                    wv_f,
                    start=True,
                    stop=True,
                )
            nc.scalar.copy(v_sb[:, b4 * 4 : (b4 + 1) * 4, :], pv)
            keeper(ctx_ps, True)
        nc.vector.tensor_scalar_add(residT, xqT_sb, rbias)

        # ---- attention ----
        ctx_sb_unused = None  # ctx read straight from PSUM in the tail
        den_all = sbBig.tile([128, NCH, CHUNK], F32)

        def attn_scores(qc, kt):
            q0 = qc * CHUNK
            k0 = kt * 128
            p_sb = [None, None]
            for g, heads in enumerate(GROUPS):
                sp = ps.tile(
                    [128, 2 * CHUNK], F32, name=f"s{g}", tag="s", bufs=3
                )
                for i, h in enumerate(heads):
                    nc.tensor.matmul(
                        sp[:, i * CHUNK : (i + 1) * CHUNK],
                        kT[h * DH : (h + 1) * DH, k0 : k0 + 128],
                        qT[h * DH : (h + 1) * DH, q0 : q0 + CHUNK],
                        start=True,
                        stop=True,
                        tile_position=(h * DH, 0),
                    )
                if g == 0:
                    pA = pPool.tile([128, 2 * CHUNK], BF16, tag="pa")
                    nc.scalar.activation(pA, sp, AF.Exp, bias=nshift, scale=1.0)
                    p_sb[0] = pA
                else:
                    pB = pPool.tile([128, 2 * CHUNK], I16, tag="pb")
                    nc.vector.tensor_scalar(pB, sp, SA, SB, op0=OP.mult, op1=OP.add)
                    p_sb[1] = pB.bitcast(BF16)
            return p_sb

        def attn_ctxden(cps, dps, kt, p_sb):
            def ctx_mms():
                for g, heads in enumerate(GROUPS):
                    for i, h in enumerate(heads):
                        nc.tensor.matmul(
                            cps[h * DH : (h + 1) * DH, :],
                            v_sb[:, kt, h * DH : (h + 1) * DH],
                            p_sb[g][:, i * CHUNK : (i + 1) * CHUNK],
                            start=(kt == 0),
                            stop=(kt == NKT - 1),
                            tile_position=(0, h * DH),
                        )

            def den_mms():
                for g, heads in enumerate(GROUPS):
                    for i, h in enumerate(heads):
                        nc.tensor.matmul(
                            dps[h * DH : (h + 1) * DH, :],
                            wones,
                            p_sb[g][:, i * CHUNK : (i + 1) * CHUNK],
                            start=(kt == 0),
                            stop=(kt == NKT - 1),
                            tile_position=(0, h * DH),
                        )

            ctx_mms()
            den_mms()

        def chunk_tail(qc, cps, halves=1):
            q0 = qc * CHUNK
            hw = CHUNK // halves
            ctxn = sbTmp.tile([128, CHUNK], F32R, tag="cn")
            out_ps = ps.tile([128, CHUNK], F32, name="out_ps", tag="s", bufs=3)
            fin = sbTmp.tile([128, CHUNK], F32, tag="fin")
            for hh in range(halves):
                sl = slice(hh * hw, (hh + 1) * hw)
                nc.vector.tensor_mul(ctxn[:, sl], cps[:, sl], den_all[:, qc, sl])
                nc.tensor.matmul(
                    out_ps[:, sl], wo_r, ctxn[:, sl], start=True, stop=True
                )
                nc.vector.tensor_add(
                    fin[:, sl], out_ps[:, sl], residT[:, q0 + hh * hw : q0 + (hh + 1) * hw]
                )
                nc.sync.dma_start(
                    out=outT_d[:, q0 + hh * hw : q0 + (hh + 1) * hw], in_=fin[:, sl]
                )

        # pre-attention warm burst (prep transposes don't register with HAM)
        for i in range(4):
            nc.tensor.matmul(ctx_ps[0:DH, :], wones, wsrc, start=True, stop=True)

        pending = attn_scores(0, 0)
        cur_ctx, cur_den = ctx_ps, den_ps
        for qc in range(NCH):
            if qc > 0:
                # allocated lazily AFTER this chunk's first scores so the
                # WAR-wait on the previous chunk's recip/ctxn readers lands
                # behind runnable PE work instead of stalling the LDW queue
                cur_ctx = ps.tile(
                    [128, CHUNK], F32, name=f"ctx{qc}", tag="ctx", bufs=1
                )
                cur_den = ps.tile(
                    [128, CHUNK], F32, name=f"den{qc}", tag="den", bufs=1
                )
            for kt in range(NKT):
                if kt + 1 < NKT:
                    nxt = attn_scores(qc, kt + 1)
                elif qc + 1 < NCH:
                    # next chunk's first scores BEFORE this chunk's tail so
                    # the PE stream has no bubble at the boundary
                    nxt = attn_scores(qc + 1, 0)
                else:
                    nxt = None
                attn_ctxden(cur_ctx, cur_den, kt, pending)
                if kt % KEEP_EVERY == KEEP_EVERY - 1 and kt < NKT - 1:
                    keeper(cur_den, False)
                pending = nxt
            if qc == NCH - 1:
                # final chunk: pipeline the tail in halves to shorten the
                # serial recip->mul->proj->add->DMA ramp-down
                nc.vector.reciprocal_approx_fast(
                    den_all[:, qc, 0:256], cur_den[:, 0:256]
                )
                nc.vector.reciprocal_approx_fast(
                    den_all[:, qc, 256:512], cur_den[:, 256:512]
                )
                chunk_tail(qc, cur_ctx, halves=2)
            else:
                nc.vector.reciprocal_approx_fast(den_all[:, qc, :], cur_den)
                chunk_tail(qc, cur_ctx)

        pPool.release()
        ps.release()
        sbTmp.release()
        sbBig.release()
        sbW.release()
        consts.release()

    nc.compile()
    return nc


def _get_compiled():
    global _compiled
    if _compiled is None:
        _compiled = _build()
    return _compiled


# device position j <- host row (j%128)*16 + j//128
_DEV2HOST = (np.arange(S) % 128) * NT + np.arange(S) // 128
_HOSTPERM = np.empty(S, dtype=np.int64)
_HOSTPERM[_DEV2HOST] = np.arange(S)


def kernel(x, Wq, bq, Wk, bk, Wv, bv, gamma, beta, Wo, bo):
    x = np.asarray(x, dtype=np.float32)
    vecs = np.stack(
        [np.asarray(a, dtype=np.float32) for a in (gamma, beta, bq, bk, bv, bo)]
    )
    wq = np.ascontiguousarray(np.asarray(Wq, dtype=np.float32))
    wk = np.ascontiguousarray(np.asarray(Wk, dtype=np.float32))
    wv = np.ascontiguousarray(np.asarray(Wv, dtype=np.float32))
    wo = np.ascontiguousarray(np.asarray(Wo, dtype=np.float32))

    nc = _get_compiled()

    in_maps = []
    for c in range(N_CORES):
        b, half = c // 2, c % 2
        off = half * QH
        xroll = np.roll(x[b], -off, axis=0)
        xin = np.ascontiguousarray(xroll[_HOSTPERM])
        xqT = np.ascontiguousarray(xroll[:QH].T)
        in_maps.append(
            {
                "xkv": xin,
                "xqT": xqT,
                "wq": wq,
                "wk": wk,
                "wv": wv,
                "wo": wo,
                "vecs": vecs,
            }
        )

    res = run_bass_kernel_spmd(nc, in_maps, core_ids=list(range(N_CORES)), trace=False)

    out = np.empty((B, S, D), dtype=np.float32)
    for c in range(N_CORES):
        b, half = c // 2, c % 2
        off = half * QH
        out[b, off : off + QH, :] = res.results[c]["outT"].T
    return out
